# revision 42
# baseline (speedup 1.0000x reference)
"""Trainium2 Bass kernel for FOAM embedding (GNN message passing).

Strategy (8 NeuronCores, SPMD, no collectives):
  - Edges sorted by edge_src; host partitions nodes into 8 contiguous
    ranges with balanced edge counts. Within a core, nodes pack into
    blocks of <=128 edges / <=6 node slots; the 128 edge slots of a
    block sit on the 128 SBUF partitions.
  - Scatter: per block one PE matmul  lhsT = Dij [128e x 128b],
    rhs = S [128e x 54],  S[e, slot*9+m] = ohw[e,slot] * Y[e,m]
    (swf = sqrt(2/rc)*switch/d folded into ohw on host).
  - Dij = senc[dst] (x) rb built on DVE at 2x via a x2-replicated senc
    (senc2[.., s, k] = senc[s], k=0,1) so every operand AP has a step-1
    innermost dim: 4 tensor_tensor ops, r-pair v per op.
  - Per-edge scalars: host sends unit vectors u = vec/d; device does
    sin/cos (ACT), the sin(n*theta) Chebyshev chain on GPSIMD, the SH
    polynomial planes on DVE (km folded in), transposing casts for
    r-inner rb and m-inner Y.  All per-granule (2 chunks) so it
    pipelines under the chunk compute.
  - Phase 3 per (chunk, l, m): two matmuls (x,y or u,v) of 336 cols;
    l0/l1: DVE multiply xl*yl straight out of PSUM; l2: scalar-engine
    Square of u=(x+y)/2, v=(x-y)/2 (sum/diff folded into the
    stationaries) then a 2x bf16 DVE subtract.  m-sums are bf16 2x DVE
    plane adds.  Outputs stream to DRAM in bf16.
"""

import os
import sys

import numpy as np

for _p in ("/opt/trn_rl_repo", "/root/.axon_site/_ro/trn_rl_repo"):
    if os.path.isdir(_p) and _p not in sys.path:
        sys.path.insert(0, _p)

import ml_dtypes  # noqa: E402

# ---------------- problem constants (hardcoded per spec) ----------------
N_RADIAL = 8
N_SPEC = 16
ZMAX = 64
CUTOFF = 5.0
NCHAN = 128
NB = N_RADIAL * N_SPEC  # 128 basis
M9 = 9                  # real SH components up to l=2

NCORES = 8
P = 128                 # edges per block == partitions
NSLOT = 6               # node slots per block
SCOLS = NSLOT * M9      # 54 moving columns per block
CH = 56                 # blocks per chunk
PSG = 8                 # blocks per scatter PSUM-copy group (x2 per tile)

BF16 = ml_dtypes.bfloat16

_COMPILED = {}
TRACE = False          # set True to capture an NTFF profile
LAST_RESULT = None     # BassKernelResults of the last kernel() call

# SH plane order: m0: 1, m1..3: x,y,z, m4: xy, m5: yz, m6: xz,
#   m7: (2z^2-x^2-y^2)/2 * sqrt5, m8: (x^2-y^2)/2 * sqrt15
_S3, _S5, _S15 = 3.0 ** 0.5, 5.0 ** 0.5, 15.0 ** 0.5


# ======================= host-side preprocessing =======================

def _partition_and_pack(edge_src, n_nodes):
    es = np.asarray(edge_src, dtype=np.int64)
    E = es.shape[0]
    deg = np.bincount(es, minlength=n_nodes)
    splits = [0]
    for c in range(1, NCORES):
        n = int(es[min((c * E) // NCORES, E - 1)])
        n = max(n, splits[-1])
        splits.append(n)
    splits.append(n_nodes)

    cores = []
    for c in range(NCORES):
        nlo, nhi = splits[c], splits[c + 1]
        blocks = []
        n = nlo
        while n < nhi:
            cnt = 0
            esum = 0
            while (n + cnt < nhi and cnt < NSLOT
                   and esum + deg[n + cnt] <= P):
                esum += deg[n + cnt]
                cnt += 1
            if cnt == 0:
                raise ValueError(
                    f"node {n} has degree {deg[n]} > {P}; unsupported")
            blocks.append((n, cnt, esum))
            n += cnt
        cores.append({"nlo": nlo, "nhi": nhi, "blocks": blocks})
    return cores, deg


def _chunk_plan(B):
    chs = []
    left = B
    while left > 0:
        c = min(CH, left)
        chs.append(c)
        left -= c
    # granules = pairs of chunks
    grans = []
    b0 = 0
    for i in range(0, len(chs), 2):
        g = sum(chs[i:i + 2])
        grans.append((b0, b0 + g))
        b0 += g
    return chs, grans


def _build_host_inputs(inputs, cores, deg, B):
    dist = np.asarray(inputs["distances"], np.float32)
    vec = np.asarray(inputs["vec"], np.float32)
    switch = np.asarray(inputs["switch"], np.float32)
    st = np.asarray(inputs["species_table"], np.float32)
    species = np.asarray(inputs["species"], np.int64)
    esrc = np.asarray(inputs["edge_src"], np.int64)
    edst = np.asarray(inputs["edge_dst"], np.int64)
    N_NODES = species.shape[0]

    senc_node = st[species]          # [N, 16]
    first_edge = np.searchsorted(esrc, np.arange(N_NODES + 1), side="left")
    bess = (2.0 / CUTOFF) ** 0.5
    swf = bess * switch / dist
    u = vec / dist[:, None]

    chs, grans = _chunk_plan(B)

    per_core = []
    for c in range(NCORES):
        blocks = cores[c]["blocks"]
        edf = np.zeros((B, 4, P), np.float32)
        edf[:, 0, :] = 1.0                      # dist pad
        edf[:, 1, :] = 1.0                      # ux pad
        senc_e = np.zeros((B, P, N_SPEC), np.float32)
        ohw = np.zeros((B, P, NSLOT), np.float32)
        slot_node = np.full((B * NSLOT,), -1, np.int64)

        for k, (n0, cnt, esum) in enumerate(blocks):
            e0 = first_edge[n0]
            e1 = first_edge[n0 + cnt]
            idx = np.arange(e0, e1)
            p = idx - e0
            edf[k, 0, p] = dist[idx]
            edf[k, 1, p] = u[idx, 0]
            edf[k, 2, p] = u[idx, 1]
            edf[k, 3, p] = u[idx, 2]
            senc_e[k, p, :] = senc_node[edst[idx]]
            loc = esrc[idx] - n0
            ohw[k, p, loc] = swf[idx]
            slot_node[k * NSLOT: k * NSLOT + cnt] = np.arange(n0, n0 + cnt)

        # edf granule-major: [P, sum_g 4*Bg]; inside granule [4, Bg]
        edf_t = edf.transpose(2, 1, 0)          # [P, 4, B]
        parts = [np.ascontiguousarray(edf_t[:, :, b0:b1]).reshape(P, -1)
                 for (b0, b1) in grans]
        edf_dev = np.concatenate(parts, axis=1)
        # senc8[p, c, s, k] = senc_e[p, c, s], k=0..7 (r-replicated)
        senc2 = np.repeat(senc_e.transpose(1, 0, 2), N_RADIAL, axis=2)
        senc2 = np.ascontiguousarray(senc2).astype(BF16)   # [P, B, 128]
        # ohm[p, c, slot, m] = ohw[p, c, slot]  (m-replicated one-hot)
        ohm = np.repeat(ohw.transpose(1, 0, 2)[:, :, :, None], M9, axis=3)
        ohm = np.ascontiguousarray(ohm).astype(BF16)       # [P, B, 6, 9]

        per_core.append(
            {
                "edf": edf_dev,
                "senc2": senc2.reshape(P, B * NB),
                "oh": ohm.reshape(P, B * SCOLS),
                "slot_node": slot_node,
            }
        )
    return per_core


def _perm_w(W):
    """Permute Dense weight rows from rs-order (r*16+s) to (s*8+r)."""
    W = np.asarray(W, np.float32)
    return np.ascontiguousarray(
        W.reshape(N_RADIAL, N_SPEC, -1).transpose(1, 0, 2).reshape(NB, -1)
    )


# ========================= device program =========================

def _build_program(B):
    import concourse.bacc as bacc
    import concourse.mybir as mybir
    import concourse.tile as tile
    from concourse.alu_op_type import AluOpType as alu

    fp32 = mybir.dt.float32
    bf16 = mybir.dt.bfloat16
    ACT = mybir.ActivationFunctionType

    chs, grans = _chunk_plan(B)
    cstart = np.cumsum([0] + chs).tolist()
    NS = NSLOT * B
    maxBg = max(b1 - b0 for b0, b1 in grans)

    nc = bacc.Bacc("TRN2", target_bir_lowering=False, debug=False,
                   num_devices=NCORES)

    edf_d = nc.dram_tensor("edf", [P, 4 * B], fp32, kind="ExternalInput")
    senc2_d = nc.dram_tensor("senc2", [P, B * NB], bf16,
                             kind="ExternalInput")
    oh_d = nc.dram_tensor("oh", [P, B * SCOLS], bf16, kind="ExternalInput")
    wx_d = nc.dram_tensor("wx", [P, 3 * NCHAN], bf16, kind="ExternalInput")
    wy_d = nc.dram_tensor("wy", [P, 3 * NCHAN], bf16, kind="ExternalInput")
    rhoi0_d = nc.dram_tensor("rhoi0", [P, NS], bf16, kind="ExternalOutput")
    xy_d = nc.dram_tensor("xy", [P, 3 * NS], bf16, kind="ExternalOutput")

    with tile.TileContext(nc) as tc:
        with (
            tc.tile_pool(name="const", bufs=1) as cpool,
            tc.tile_pool(name="edfp", bufs=2) as edfpool,
            tc.tile_pool(name="pha", bufs=2) as papool,
            tc.tile_pool(name="rbt", bufs=1) as rbtpool,
            tc.tile_pool(name="chunk", bufs=2) as ckpool,
            tc.tile_pool(name="big", bufs=1) as bigpool,
            tc.tile_pool(name="p3", bufs=2) as p3pool,
            tc.tile_pool(name="ps_sc", bufs=2, space="PSUM") as pssc,
            tc.tile_pool(name="ps_p3", bufs=2, space="PSUM") as psp,
        ):
            # wx/wy cols: l0,l1 raw; l2 replaced by u=(wx+wy)/2, v=(wx-wy)/2
            wx = cpool.tile([P, 3 * NCHAN], bf16, tag="wx")
            wy = cpool.tile([P, 3 * NCHAN], bf16, tag="wy")
            # wx/wy DMAs are deferred until after phase_a(0) so the
            # critical-path edf DMA heads the queues
            half_pi = cpool.tile([P, 1], fp32, tag="halfpi")
            nc.vector.memset(half_pi[:], float(np.pi / 2))

            rb_t = rbtpool.tile([P, B * N_RADIAL], bf16, tag="rbt")
            Y = rbtpool.tile([P, B * M9], bf16, tag="Y")
            rhoi_sb = bigpool.tile([P, SCOLS * B], bf16, tag="rhoi")

            def phase_a(gi):
                b0, b1 = grans[gi]
                Bg = b1 - b0
                edf = edfpool.tile([P, 4 * maxBg], fp32, tag="edf")
                nc.sync.dma_start(out=edf[:, 0:4 * Bg],
                                  in_=edf_d[:, 4 * b0:4 * b1])
                d_ap = edf[:, 0:Bg]
                u_ap = edf[:, Bg:4 * Bg]
                ux, uy, uz = (edf[:, (1 + i) * Bg:(2 + i) * Bg]
                              for i in range(3))

                # radial sin(n*theta) via Chebyshev on GPSIMD
                rbp = papool.tile([P, N_RADIAL * maxBg], fp32, tag="rbp")
                cosd = papool.tile([P, maxBg], fp32, tag="cosd")
                nc.scalar.activation(
                    out=rbp[:, 0:Bg], in_=d_ap, func=ACT.Sin,
                    scale=float(np.pi / CUTOFF))
                nc.scalar.activation(
                    out=cosd[:, 0:Bg], in_=d_ap, func=ACT.Sin,
                    scale=float(-np.pi / CUTOFF), bias=half_pi[:])
                nc.vector.tensor_scalar(
                    out=cosd[:, 0:Bg], in0=cosd[:, 0:Bg], scalar1=2.0,
                    scalar2=None, op0=alu.mult)
                nc.vector.tensor_tensor(
                    out=rbp[:, maxBg:maxBg + Bg], in0=cosd[:, 0:Bg],
                    in1=rbp[:, 0:Bg], op=alu.mult)
                for n in range(2, N_RADIAL):
                    nc.vector.tensor_tensor(
                        out=rbp[:, n * maxBg:n * maxBg + Bg],
                        in0=cosd[:, 0:Bg],
                        in1=rbp[:, (n - 1) * maxBg:(n - 1) * maxBg + Bg],
                        op=alu.mult)
                    nc.vector.tensor_tensor(
                        out=rbp[:, n * maxBg:n * maxBg + Bg],
                        in0=rbp[:, n * maxBg:n * maxBg + Bg],
                        in1=rbp[:, (n - 2) * maxBg:(n - 2) * maxBg + Bg],
                        op=alu.subtract)
                # transposing cast to r-inner bf16 (DVE)
                nc.vector.tensor_copy(
                    out=rb_t[:, b0 * N_RADIAL:b1 * N_RADIAL].rearrange(
                        "p (c n) -> p c n", n=N_RADIAL),
                    in_=rbp[:].rearrange("p (n c) -> p n c", n=N_RADIAL)
                        [:, :, 0:Bg].transpose([0, 2, 1]),
                )

                # SH planes m-major fp32, km folded
                sq = papool.tile([P, 3 * maxBg], fp32, tag="sq")
                nc.vector.tensor_tensor(out=sq[:, 0:3 * Bg], in0=u_ap,
                                        in1=u_ap, op=alu.mult)
                sqx = sq[:, 0:Bg]
                sqy = sq[:, Bg:2 * Bg]
                sqz = sq[:, 2 * Bg:3 * Bg]
                ab2 = papool.tile([P, maxBg], fp32, tag="ab2")
                nc.vector.tensor_tensor(out=ab2[:, 0:Bg], in0=sqx, in1=sqy,
                                        op=alu.add)
                Yp = papool.tile([P, M9 * maxBg], fp32, tag="Yp")

                def ypl(m, n=1):
                    return Yp[:, m * maxBg:m * maxBg + n * Bg] \
                        if n == 1 else Yp[:, m * maxBg:(m + n - 1) * maxBg + Bg]

                nc.vector.memset(Yp[:, 0:Bg], 1.0)
                # m1..3 = sqrt3 * u   (single op over 3 planes)
                nc.vector.tensor_scalar(
                    out=Yp[:].rearrange("p (m c) -> p m c", m=M9)
                        [:, 1:4, 0:Bg],
                    in0=u_ap.rearrange("p (t c) -> p t c", t=3),
                    scalar1=_S3, scalar2=None, op0=alu.mult)
                # m4 = x*y, m5 = y*z  (pair), m6 = x*z; one *s15 pass after
                nc.vector.tensor_tensor(
                    out=Yp[:].rearrange("p (m c) -> p m c", m=M9)
                        [:, 4:6, 0:Bg],
                    in0=edf[:, 0:4 * Bg].rearrange("p (t c) -> p t c", t=4)
                        [:, 1:3, :],
                    in1=edf[:, 0:4 * Bg].rearrange("p (t c) -> p t c", t=4)
                        [:, 2:4, :],
                    op=alu.mult)
                nc.vector.tensor_tensor(out=ypl(6), in0=ux, in1=uz,
                                        op=alu.mult)
                nc.vector.tensor_scalar(
                    out=Yp[:, 4 * maxBg:6 * maxBg + Bg],
                    in0=Yp[:, 4 * maxBg:6 * maxBg + Bg],
                    scalar1=_S15, scalar2=None, op0=alu.mult)
                # m7 = s5 * (2 z^2 - (x^2+y^2)) / 2 = (sqz*2 - ab2)*s5/2
                nc.vector.scalar_tensor_tensor(
                    out=ypl(7), in0=sqz, scalar=2.0, in1=ab2[:, 0:Bg],
                    op0=alu.mult, op1=alu.subtract)
                nc.vector.tensor_scalar(
                    out=ypl(7), in0=ypl(7), scalar1=0.5 * _S5, scalar2=None,
                    op0=alu.mult)
                # m8 = (x^2-y^2)*s15/2
                nc.vector.tensor_tensor(out=ypl(8), in0=sqx, in1=sqy,
                                        op=alu.subtract)
                nc.vector.tensor_scalar(
                    out=ypl(8), in0=ypl(8), scalar1=0.5 * _S15, scalar2=None,
                    op0=alu.mult)
                # m-inner bf16 cast (DVE; strided read)
                nc.vector.tensor_copy(
                    out=Y[:, b0 * M9:b1 * M9].rearrange(
                        "p (c m) -> p c m", m=M9),
                    in_=Yp[:].rearrange("p (m c) -> p m c", m=M9)
                        [:, :, 0:Bg].transpose([0, 2, 1]),
                )

            def chunk_scatter(ci):
                ch = chs[ci]
                c0 = cstart[ci]
                slotc = ch * NSLOT
                base = c0 * NSLOT
                senc2 = ckpool.tile([P, CH * NB], bf16, tag="senc2")
                oh = ckpool.tile([P, CH * SCOLS], bf16, tag="oh")
                nc.sync.dma_start(
                    out=senc2[:, 0:ch * NB],
                    in_=senc2_d[:, c0 * NB:(c0 + ch) * NB])
                nc.sync.dma_start(
                    out=oh[:, 0:ch * SCOLS],
                    in_=oh_d[:, c0 * SCOLS:(c0 + ch) * SCOLS])

                # S[p, c, slot, m] = ohm[p, c, slot, m] * Y[p, c, m]
                # (2x DVE: step-1 innermost on all operands)
                S = ckpool.tile([P, CH * SCOLS], bf16, tag="S")
                nc.vector.tensor_tensor(
                    out=S[:, 0:ch * SCOLS].rearrange(
                        "p (c l m) -> p c l m", l=NSLOT, m=M9),
                    in0=oh[:, 0:ch * SCOLS].rearrange(
                        "p (c l m) -> p c l m", l=NSLOT, m=M9),
                    in1=Y[:, c0 * M9:(c0 + ch) * M9]
                        .rearrange("p (c m) -> p c m", m=M9)
                        .unsqueeze(2).broadcast_to([P, ch, NSLOT, M9]),
                    op=alu.mult,
                )
                # Dij[p, c, s, r] = senc8[p, c, s, r] * rb_t[p, c, r]
                Dij = ckpool.tile([P, CH * NB], bf16, tag="Dij")
                nc.vector.tensor_tensor(
                    out=Dij[:, 0:ch * NB].rearrange(
                        "p (c s r) -> p c s r", s=N_SPEC, r=N_RADIAL),
                    in0=senc2[:, 0:ch * NB].rearrange(
                        "p (c s r) -> p c s r", s=N_SPEC, r=N_RADIAL),
                    in1=rb_t[:, c0 * N_RADIAL:(c0 + ch) * N_RADIAL]
                        .rearrange("p (c r) -> p c r", r=N_RADIAL)
                        .unsqueeze(2).broadcast_to([P, ch, N_SPEC, N_RADIAL]),
                    op=alu.mult,
                )

                # scatter matmuls; 2*PSG blocks per 2-bank PSUM tile.
                # rhoi_sb is m-plane-major [P, M9*NS]: the scatter copy
                # transposes (blk, slot, m) -> (m, blk, slot) so phase-3
                # moving operands are slot-contiguous.
                rho_v = rhoi_sb[:].rearrange("p (m s) -> p m s", s=NS)
                for g in range(ch // PSG):
                    pst = pssc.tile([P, PSG * SCOLS], fp32, tag="psc")
                    for j in range(PSG):
                        k = g * PSG + j
                        nc.tensor.matmul(
                            out=pst[:, j * SCOLS:(j + 1) * SCOLS],
                            lhsT=Dij[:, k * NB:(k + 1) * NB],
                            rhs=S[:, k * SCOLS:(k + 1) * SCOLS],
                            start=True, stop=True,
                        )
                    s0 = (c0 + g * PSG) * NSLOT
                    cp = nc.scalar.copy
                    kw = {"in_": pst[:].rearrange(
                        "p (b l m) -> p b l m", l=NSLOT, m=M9)
                        .transpose([0, 3, 1, 2])}
                    cp(out=rho_v[:, :, s0:s0 + PSG * NSLOT].rearrange(
                        "p m (b l) -> p m b l", l=NSLOT), **kw)
                # rhoi0 output = the m0 plane, direct DMA
                nc.sync.dma_start(out=rhoi0_d[:, base:base + slotc],
                                  in_=rhoi_sb[:, base:base + slotc])

            def chunk_p3(ci):
                ch = chs[ci]
                c0 = cstart[ci]
                slotc = ch * NSLOT
                base = c0 * NSLOT
                rho_v = rhoi_sb[:].rearrange("p (m s) -> p m s", s=NS)
                # (l, m-range) groups; each group: batched x/y matmuls over
                # contiguous slot columns, scalar-staged to SBUF bf16, one
                # 2x DVE multiply
                groups = [(0, 0, 1), (1, 1, 3), (2, 4, 2), (2, 6, 3)]
                txys = {}
                for l, mlo, mn in groups:
                    txy = txys.get(l)
                    if txy is None:
                        txy = p3pool.tile([P, (2 * l + 1) * CH * NSLOT],
                                          bf16, tag=f"txy{l}")
                        txys[l] = txy
                    cols = mn * slotc
                    xsb = p3pool.tile([P, 3 * CH * NSLOT], bf16, tag="xsb")
                    ysb = p3pool.tile([P, 3 * CH * NSLOT], bf16, tag="ysb")
                    # per-m sub-matmuls into bank-aligned 512-col slots
                    # (one matmul may not cross a PSUM bank); one batched
                    # scalar copy per side
                    for side, w_ap, sb in ((0, wx, xsb), (1, wy, ysb)):
                        pp = psp.tile([P, 3 * 512], fp32, tag="pp")
                        for mi in range(mn):
                            nc.tensor.matmul(
                                out=pp[:, mi * 512:mi * 512 + slotc],
                                lhsT=w_ap[:, l * NCHAN:(l + 1) * NCHAN],
                                rhs=rho_v[:, mlo + mi,
                                          base:base + slotc],
                                start=True, stop=True)
                        nc.scalar.copy(
                            out=sb[:, 0:cols].rearrange(
                                "p (m s) -> p m s", m=mn),
                            in_=pp[:].rearrange("p (m s) -> p m s", s=512)
                                [:, 0:mn, 0:slotc],
                        )
                    toff = (mlo - l * l) * slotc
                    nc.vector.tensor_tensor(
                        out=txy[:, toff:toff + cols], in0=xsb[:, 0:cols],
                        in1=ysb[:, 0:cols], op=alu.mult)
                for l in range(3):
                    txy = txys[l]
                    # m-sum (bf16 2x plane adds) + store
                    if l == 0:
                        out_ap = txy[:, 0:slotc]
                    elif l == 1:
                        t01 = p3pool.tile([P, CH * NSLOT], bf16, tag="t01")
                        nc.vector.tensor_tensor(
                            out=t01[:, 0:slotc], in0=txy[:, 0:slotc],
                            in1=txy[:, slotc:2 * slotc], op=alu.add)
                        xyt = p3pool.tile([P, CH * NSLOT], bf16, tag="xyt1")
                        nc.vector.tensor_tensor(
                            out=xyt[:, 0:slotc], in0=t01[:, 0:slotc],
                            in1=txy[:, 2 * slotc:3 * slotc], op=alu.add)
                        out_ap = xyt[:, 0:slotc]
                    else:
                        ab = p3pool.tile([P, 2 * CH * NSLOT], bf16, tag="abt")
                        nc.vector.tensor_tensor(
                            out=ab[:, 0:2 * slotc], in0=txy[:, 0:2 * slotc],
                            in1=txy[:, 2 * slotc:4 * slotc], op=alu.add)
                        cde = p3pool.tile([P, CH * NSLOT], bf16, tag="cde")
                        nc.vector.tensor_tensor(
                            out=cde[:, 0:slotc], in0=ab[:, 0:slotc],
                            in1=ab[:, slotc:2 * slotc], op=alu.add)
                        xyt = p3pool.tile([P, CH * NSLOT], bf16, tag="xyt2")
                        nc.vector.tensor_tensor(
                            out=xyt[:, 0:slotc], in0=cde[:, 0:slotc],
                            in1=txy[:, 4 * slotc:5 * slotc], op=alu.add)
                        out_ap = xyt[:, 0:slotc]
                    nc.sync.dma_start(
                        out=xy_d[:, l * NS + base:l * NS + base + slotc],
                        in_=out_ap)

            # software pipeline with 1-chunk lag: phase 3 of chunk c-1
            # runs (on PE/SC/DVE) while chunk c's scatter is in flight
            for gi in range(len(grans)):
                phase_a(gi)
                if gi == 0:
                    nc.sync.dma_start(out=wx[:], in_=wx_d[:])
                    nc.sync.dma_start(out=wy[:], in_=wy_d[:])
                for ci in (2 * gi, 2 * gi + 1):
                    if ci < len(chs):
                        chunk_scatter(ci)
                        if ci > 0:
                            chunk_p3(ci - 1)
            chunk_p3(len(chs) - 1)

    nc.finalize()
    return nc


# ============================ entry point ============================

def kernel(**inputs):
    from concourse.bass_utils import run_bass_kernel_spmd

    species = np.asarray(inputs["species"], np.int64)
    N_NODES = species.shape[0]
    cores, deg = _partition_and_pack(np.asarray(inputs["edge_src"]), N_NODES)
    maxb = max(len(c["blocks"]) for c in cores)
    B = ((maxb + PSG - 1) // PSG) * PSG
    NS = NSLOT * B

    per_core = _build_host_inputs(inputs, cores, deg, B)

    wx = np.empty((P, 3 * NCHAN), np.float32)
    wy = np.empty((P, 3 * NCHAN), np.float32)
    for l, key in enumerate(("W0", "W1", "W2")):
        Wp = _perm_w(inputs[key])
        wx[:, l * NCHAN:(l + 1) * NCHAN] = Wp[:, :NCHAN]
        wy[:, l * NCHAN:(l + 1) * NCHAN] = (
            Wp[:, NCHAN:] / np.sqrt(2 * l + 1.0))
    wx = wx.astype(BF16)
    wy = wy.astype(BF16)

    if B not in _COMPILED:
        _COMPILED[B] = _build_program(B)
    nc = _COMPILED[B]

    in_maps = [
        {"edf": pc["edf"], "senc2": pc["senc2"], "oh": pc["oh"],
         "wx": wx, "wy": wy}
        for pc in per_core
    ]
    res = run_bass_kernel_spmd(nc, in_maps, list(range(NCORES)),
                               trace=TRACE)
    global LAST_RESULT
    LAST_RESULT = res

    # ---------------- host assembly ----------------
    st = np.asarray(inputs["species_table"], np.float32)
    out = np.zeros((N_NODES, N_SPEC + NB + 3 * NCHAN), np.float32)
    out[:, :N_SPEC] = st[species]

    r = np.arange(NB) // N_SPEC
    s = np.arange(NB) % N_SPEC
    dev_of_rs = s * N_RADIAL + r

    for c in range(NCORES):
        sn = per_core[c]["slot_node"]
        valid = sn >= 0
        nodes = sn[valid]
        slots = np.nonzero(valid)[0]
        r0 = np.asarray(per_core_res(res, c, "rhoi0"), np.float32)
        xy = np.asarray(per_core_res(res, c, "xy"), np.float32)
        out[nodes, N_SPEC:N_SPEC + NB] = r0[dev_of_rs][:, slots].T
        for l in range(3):
            out[nodes,
                N_SPEC + NB + l * NCHAN:N_SPEC + NB + (l + 1) * NCHAN] = (
                xy[:, l * NS + slots].T)
    return out


def per_core_res(res, c, name):
    return res.results[c][name]
